# revision 2
# baseline (speedup 1.0000x reference)
"""IPA forward, Bass/Tile, 8 TRN2 cores — transposed-logits v3.

lgT[j,(q,h)] per 32-query superblock; bias via per-query bf16 matmuls off
the DMA'd zT-layout slab; softmax unnormalized, normalization folded into
output extraction (epi: ones-column sums; pairT: PE colsum -> reciprocal ->
partition_broadcast); PE transposes give [j,d] z for out_pair.
"""
import sys
sys.path.insert(0, '/opt/trn_rl_repo')

import math
import numpy as np

import concourse.bass as bass
import concourse.tile as tile
from concourse import bacc, mybir
from concourse.masks import make_identity

C_S, C_Z = 384, 128
H, D = 12, 16
PQ, PV = 4, 8
B, N = 2, 512
EPS = 1e-8

N_CORES = 8
NI = 128
SBQ = 32            # queries per superblock
NSB = NI // SBQ
NZT = 8             # z DMA tiles (16 queries each)

SCALE_QK = math.sqrt(1.0 / (3.0 * D))
SCALE_B = math.sqrt(1.0 / 3.0)
WC = math.sqrt(1.0 / (3.0 * (PQ * 9.0 / 2.0)))

PROJ_J = 816
OFF_K, OFF_V, OFF_KP, OFF_VP = 0, 192, 384, 528
PROJ_Q = 336
OFF_Q, OFF_QP = 0, 192
EW = 41             # epi cols per head: 16 v | 24 vg | 1 ones


def softplus(x):
    return np.logaddexp(0.0, x)


def pack_weights(inp):
    import ml_dtypes
    wq, bq = np.asarray(inp['wq']), np.asarray(inp['bq'])
    wkv, bkv = np.asarray(inp['wkv']), np.asarray(inp['bkv'])
    wqp, bqp = np.asarray(inp['wqp']), np.asarray(inp['bqp'])
    wkvp, bkvp = np.asarray(inp['wkvp']), np.asarray(inp['bkvp'])
    wb = np.asarray(inp['wb'])
    wout, bout = np.asarray(inp['wout']), np.asarray(inp['bout'])
    head_weights = np.asarray(inp['head_weights'])

    wcat_j = np.zeros((C_S, PROJ_J), np.float32)
    bcat_j = np.zeros((PROJ_J,), np.float32)
    wcat_q = np.zeros((C_S, PROJ_Q), np.float32)
    bcat_q = np.zeros((PROJ_Q,), np.float32)

    kv = wkv.reshape(C_S, H, 2 * D)
    bkv2 = bkv.reshape(H, 2 * D)
    wcat_j[:, OFF_K:OFF_K + 192] = kv[:, :, :D].reshape(C_S, 192)
    bcat_j[OFF_K:OFF_K + 192] = bkv2[:, :D].reshape(192)
    wcat_j[:, OFF_V:OFF_V + 192] = kv[:, :, D:].reshape(C_S, 192)
    bcat_j[OFF_V:OFF_V + 192] = bkv2[:, D:].reshape(192)

    kvp = wkvp.reshape(C_S, 3, H, PQ + PV)
    bkvp2 = bkvp.reshape(3, H, PQ + PV)
    wcat_j[:, OFF_KP:OFF_KP + 144] = kvp[:, :, :, :PQ].reshape(C_S, 144)
    bcat_j[OFF_KP:OFF_KP + 144] = bkvp2[:, :, :PQ].reshape(144)
    wcat_j[:, OFF_VP:OFF_VP + 288] = kvp[:, :, :, PQ:].reshape(C_S, 288)
    bcat_j[OFF_VP:OFF_VP + 288] = bkvp2[:, :, PQ:].reshape(288)

    wcat_q[:, OFF_Q:OFF_Q + 192] = wq
    bcat_q[OFF_Q:OFF_Q + 192] = bq
    wcat_q[:, OFF_QP:OFF_QP + 144] = wqp
    bcat_q[OFF_QP:OFF_QP + 144] = bqp

    hw = (softplus(head_weights) * WC).astype(np.float32)

    wbs = (wb * SCALE_B).astype(np.float32)

    qsel = np.zeros((3, 128, H), np.float32)
    for h in range(H):
        for dd in range(D):
            g = h * D + dd
            qsel[g // 128, g % 128, h] = SCALE_QK
    for c in range(3):
        for h in range(H):
            for q in range(PQ):
                g = c * 48 + h * 4 + q
                if g < 64:
                    ch, r = 1, 64 + g
                elif g < 128:
                    ch, r = 2, g - 64
                else:
                    ch, r = 2, 64 + (g - 128)
                qsel[ch, r, h] = hw[h]
    for h in range(H):
        qsel[2, 96 + h, h] = -0.5 * hw[h]

    wout_dev = np.zeros((18, 128, C_S), np.float32)
    wo = wout
    wout_dev[0] = wo[0:128]
    wout_dev[1, :64] = wo[128:192]
    wout_dev[2, :96] = wo[192:288]
    wout_dev[3, :96] = wo[288:384]
    wout_dev[4, :96] = wo[384:480]
    wout_dev[5, :96] = wo[480:576]
    for h in range(H):
        wout_dev[6 + h] = wo[576 + h * C_Z: 576 + (h + 1) * C_Z]
    # [128, 18*384] bf16, single DMA
    wout_flat = np.ascontiguousarray(
        wout_dev.transpose(1, 0, 2).reshape(128, 18 * C_S)).astype(ml_dtypes.bfloat16)

    bf = ml_dtypes.bfloat16
    return dict(
        wcat_j=np.ascontiguousarray(
            wcat_j.reshape(3, 128, PROJ_J).transpose(1, 0, 2).reshape(128, 3 * PROJ_J)
        ).astype(bf),
        bcat_j=bcat_j.reshape(1, PROJ_J),
        wcat_q=np.ascontiguousarray(
            wcat_q.reshape(3, 128, PROJ_Q).transpose(1, 0, 2).reshape(128, 3 * PROJ_Q)
        ).astype(bf),
        bcat_q=bcat_q.reshape(1, PROJ_Q),
        wbs=wbs,
        qsel=np.ascontiguousarray(qsel.transpose(1, 0, 2).reshape(128, 3 * H)),
        wout_flat=wout_flat,
        bout=bout.reshape(1, C_S).astype(np.float32),
    )


def per_core_inputs(inp, packed, core):
    import ml_dtypes
    bf = ml_dtypes.bfloat16
    b = core // 4
    i0 = (core % 4) * NI
    s = np.asarray(inp['single_representation'])[b]
    z = np.asarray(inp['pair_representation'])[b]
    R = np.asarray(inp['rotations'])[b]
    t = np.asarray(inp['translation'])[b]

    sT3 = s.T.reshape(3, 128, N)
    sT = np.ascontiguousarray(sT3.transpose(1, 0, 2).reshape(128, 3 * N)).astype(bf)
    sTq = np.ascontiguousarray(
        sT3[:, :, i0:i0 + NI].transpose(1, 0, 2).reshape(128, 3 * NI)).astype(bf)
    Rt = np.concatenate([R.reshape(N, 9), t], axis=1).astype(np.float32)
    Rt4 = np.ascontiguousarray(Rt.reshape(4, 128, 12).transpose(1, 0, 2).reshape(128, 48))

    zs = z[i0:i0 + NI]                                     # [128, 512, 128]
    zt = zs.reshape(NZT, 16, N, C_Z).transpose(0, 3, 1, 2) # [8, 128, 16, 512]
    zt = np.ascontiguousarray(zt.reshape(NZT, C_Z, 16 * N)).astype(bf)

    return {
        'sT': sT, 'sTq': sTq, 'zt': zt,
        'Rt': Rt4,
        'Rt_q': np.ascontiguousarray(Rt[i0:i0 + NI]),
        'wcat_j': packed['wcat_j'], 'bcat_j': packed['bcat_j'],
        'wcat_q': packed['wcat_q'], 'bcat_q': packed['bcat_q'],
        'wbs': packed['wbs'], 'qsel': packed['qsel'],
        'wout_flat': packed['wout_flat'], 'bout': packed['bout'],
    }, b, i0


F32 = mybir.dt.float32
F32R = mybir.dt.float32r
BF16 = mybir.dt.bfloat16
AF = mybir.ActivationFunctionType
ALU = mybir.AluOpType
AX = mybir.AxisListType


def r32(x):
    return x.bitcast(F32R)


def build_kernel(repeat=1, dbg=False):
    nc = bacc.Bacc("TRN2", target_bir_lowering=False, debug=False,
                   num_devices=N_CORES)
    p = {}
    if dbg:
        p['dbg_qexp0'] = nc.declare_dram_parameter("dbg_qexp0", [128, NI * H], F32, isOutput=True)
        p['dbg_kf0'] = nc.declare_dram_parameter("dbg_kf0", [128, N], F32, isOutput=True)
        p['dbg_kf2'] = nc.declare_dram_parameter("dbg_kf2", [128, N], F32, isOutput=True)
        p['dbg_at0'] = nc.declare_dram_parameter("dbg_at0", [128, SBQ * H], F32, isOutput=True)
        p['dbg_epi0'] = nc.declare_dram_parameter("dbg_epi0", [32, H * EW], F32, isOutput=True)
        p['dbg_op0'] = nc.declare_dram_parameter("dbg_op0", [128, SBQ * H], F32, isOutput=True)
        p['dbg_rcpb'] = nc.declare_dram_parameter("dbg_rcpb", [128, SBQ * H], F32, isOutput=True)
        p['dbg_pairT'] = nc.declare_dram_parameter("dbg_pairT", [128, H * 128], F32, isOutput=True)
        p['dbg_sa'] = nc.declare_dram_parameter("dbg_sa", [128, H * D], F32, isOutput=True)
        p['dbg_rpg'] = nc.declare_dram_parameter("dbg_rpg", [128, 3 * 96], F32, isOutput=True)
        p['dbg_znat0'] = nc.declare_dram_parameter("dbg_znat0", [128, 2 * N], F32, isOutput=True)
    p['sT'] = nc.declare_dram_parameter("sT", [128, 3 * N], BF16, isOutput=False)
    p['sTq'] = nc.declare_dram_parameter("sTq", [128, 3 * NI], BF16, isOutput=False)
    p['z'] = nc.declare_dram_parameter("zt", [NZT, C_Z, 16 * N], BF16, isOutput=False)
    p['Rt'] = nc.declare_dram_parameter("Rt", [128, 48], F32, isOutput=False)
    p['Rtq'] = nc.declare_dram_parameter("Rt_q", [NI, 12], F32, isOutput=False)
    p['wcat_j'] = nc.declare_dram_parameter("wcat_j", [128, 3 * PROJ_J], BF16, isOutput=False)
    p['bcat_j'] = nc.declare_dram_parameter("bcat_j", [1, PROJ_J], F32, isOutput=False)
    p['wcat_q'] = nc.declare_dram_parameter("wcat_q", [128, 3 * PROJ_Q], BF16, isOutput=False)
    p['bcat_q'] = nc.declare_dram_parameter("bcat_q", [1, PROJ_Q], F32, isOutput=False)
    p['wbs'] = nc.declare_dram_parameter("wbs", [C_Z, H], F32, isOutput=False)
    p['qsel'] = nc.declare_dram_parameter("qsel", [128, 3 * H], F32, isOutput=False)
    p['wout'] = nc.declare_dram_parameter("wout_flat", [128, 18 * C_S], BF16, isOutput=False)
    p['bout'] = nc.declare_dram_parameter("bout", [1, C_S], F32, isOutput=False)
    p['out'] = nc.declare_dram_parameter("out", [NI, C_S], F32, isOutput=True)

    with tile.TileContext(nc) as tc:
        if repeat > 1:
            with tc.For_i(0, repeat, 1):
                _body(nc, tc, p)
        else:
            _body(nc, tc, p)
    nc.compile()
    return nc


def _body(nc, tc, p):
    dma = nc.sync.dma_start

    pers_cm = tc.tile_pool(name="pers", bufs=1)
    pers = pers_cm.__enter__()
    # z pool opened top-level so its SBUF never aliases phase-0 pools and
    # the prefetch DMAs don't wait on phase-0 consumers.
    zp_cm = tc.tile_pool(name="zp", bufs=4)
    zp = zp_cm.__enter__()

    ident = pers.tile([128, 128], F32)
    make_identity(nc, ident[:])
    ident_bf = pers.tile([128, 128], BF16)
    nc.vector.tensor_copy(ident_bf[:], ident[:])
    ident_r = pers.tile([128, 128], F32R)
    nc.gpsimd.tensor_copy(ident_r[:], ident[:])
    ones_bf = pers.tile([128, 1], BF16)
    nc.gpsimd.memset(ones_bf[:], 1.0)
    ones1_bf = pers.tile([1, 128], BF16)
    nc.gpsimd.memset(ones1_bf[:], 1.0)
    ones1 = pers.tile([1, 128], F32)
    nc.gpsimd.memset(ones1[:], 1.0)

    wbs_bf = pers.tile([C_Z, H], BF16)
    Rt_sb = pers.tile([128, 48], F32)
    Rtq_sb = pers.tile([128, 12], F32)

    kfullT = [pers.tile([128, N], F32R, tag=f"kfullT{c}", name=f"kfullT{c}")
              for c in range(3)]
    nc.gpsimd.memset(kfullT[2][:].bitcast(F32), 0.0)
    qexp = [pers.tile([128, NI * H], F32R, tag=f"qexp{c}", name=f"qexp{c}")
            for c in range(3)]
    vvg_bf = pers.tile([128, 4 * H * EW], BF16)
    pairT = pers.tile([128, H * 128], BF16)
    scalar_all = pers.tile([128, H * D], BF16)
    rpg_all = pers.tile([128, 3 * 96], F32)

    # ---- DMAs: phase-0 weights first, then the z slab ----
    sT_sb = pers.tile([128, 3 * N], BF16)
    dma(sT_sb[:], p['sT'][:])
    sTq_sb = pers.tile([128, 3 * NI], BF16)
    dma(sTq_sb[:], p['sTq'][:])
    wcj_sb = pers.tile([128, 3 * PROJ_J], BF16)
    dma(wcj_sb[:], p['wcat_j'][:])
    wcq_sb = pers.tile([128, 3 * PROJ_Q], BF16)
    dma(wcq_sb[:], p['wcat_q'][:])
    bcj_sb = pers.tile([1, PROJ_J], F32)
    dma(bcj_sb[:], p['bcat_j'][:])
    bcq_sb = pers.tile([1, PROJ_Q], F32)
    dma(bcq_sb[:], p['bcat_q'][:])
    qsel_sb = pers.tile([128, 3 * H], F32)
    dma(qsel_sb[:], p['qsel'][:])
    wbs_f = pers.tile([C_Z, H], F32)
    dma(wbs_f[:], p['wbs'][:])
    nc.vector.tensor_copy(wbs_bf[:], wbs_f[:])
    dma(Rt_sb[:], p['Rt'][:])
    dma(Rtq_sb[:], p['Rtq'][:])

    z_tiles = []
    for tzi in range(NZT):
        zt_sb = zp.tile([128, 16 * N], BF16, tag="z", name="z_sb")
        dma(zt_sb[:], p['z'][tzi])
        z_tiles.append(zt_sb)

    # =================== PHASE 0 ===================
    with tc.tile_pool(name="ph0", bufs=1) as ph0, \
         tc.tile_pool(name="ph0b", bufs=2) as ph0b, \
         tc.tile_pool(name="ph0ps", bufs=2, space="PSUM") as ph0ps:
        proj_nat = ph0.tile([128, 4 * PROJ_J], F32R)
        projq = ph0.tile([128, PROJ_Q], F32R)
        strips = [(0, 384), (384, 432)]
        for jc in range(4):
            for si, (s0, sw) in enumerate(strips):
                pp = ph0ps.tile([128, 432], F32, tag="projps", name="pp")
                for kc in range(3):
                    nc.tensor.matmul(
                        pp[:, 0:sw],
                        sT_sb[:, kc * N + jc * 128: kc * N + (jc + 1) * 128],
                        wcj_sb[:, kc * PROJ_J + s0: kc * PROJ_J + s0 + sw],
                        start=(kc == 0), stop=False)
                nc.tensor.matmul(pp[:, 0:sw], ones1[:], bcj_sb[:, s0:s0 + sw],
                                 start=False, stop=True)
                dcol = jc * PROJ_J + s0
                if (jc + si) % 2 == 0:
                    nc.vector.tensor_copy(proj_nat[:, dcol:dcol + sw], pp[:, 0:sw])
                else:
                    nc.scalar.copy(proj_nat[:, dcol:dcol + sw], pp[:, 0:sw])
        ppq = ph0ps.tile([128, 432], F32, tag="projps", name="ppq")
        for kc in range(3):
            nc.tensor.matmul(
                ppq[:, 0:PROJ_Q],
                sTq_sb[:, kc * NI:(kc + 1) * NI],
                wcq_sb[:, kc * PROJ_Q:(kc + 1) * PROJ_Q],
                start=(kc == 0), stop=False)
        nc.tensor.matmul(ppq[:, 0:PROJ_Q], ones1[:], bcq_sb[:],
                         start=False, stop=True)
        nc.vector.tensor_copy(projq[:], ppq[:, 0:PROJ_Q])

        kg_nat = ph0.tile([128, 4 * 144], F32R)
        vg_nat = ph0.tile([128, 4 * 288], F32)
        qg_nat = ph0.tile([128, 144], F32R)

        def glob_points(dst, dcol0, src, scol0, bw, n_rt, rt_tile):
            # d_c = src_0*R[c,0] + t_c (Act), then += src_p*R[c,p] (DVE/Pool)
            for cp in range(3):
                d = dst[:, dcol0 + cp * bw: dcol0 + (cp + 1) * bw]
                nc.scalar.activation(
                    d, src[:, scol0:scol0 + bw].bitcast(F32), AF.Identity,
                    bias=rt_tile[:, n_rt + 9 + cp: n_rt + 9 + cp + 1],
                    scale=rt_tile[:, n_rt + cp * 3: n_rt + cp * 3 + 1])
                for pp_, eng in ((1, nc.vector), (2, nc.vector)):
                    eng.scalar_tensor_tensor(
                        d, src[:, scol0 + pp_ * bw: scol0 + (pp_ + 1) * bw].bitcast(F32),
                        rt_tile[:, n_rt + cp * 3 + pp_: n_rt + cp * 3 + pp_ + 1],
                        d.bitcast(F32),
                        op0=ALU.mult, op1=ALU.add)

        for jc in range(4):
            glob_points(kg_nat, jc * 144, proj_nat,
                        jc * PROJ_J + OFF_KP, 48, jc * 12, Rt_sb)
        glob_points(qg_nat, 0, projq, OFF_QP, 48, 0, Rtq_sb)
        for jc in range(4):
            glob_points(vg_nat, jc * 288, proj_nat,
                        jc * PROJ_J + OFF_VP, 96, jc * 12, Rt_sb)

        # vvg (v | vg | ones), bf16, [j, (h, 41)]
        for jc in range(4):
            base = jc * H * EW
            nc.scalar.copy(
                vvg_bf[:, base:base + H * EW]
                .rearrange("p (h w) -> p h w", h=H)[:, :, 0:D],
                proj_nat[:, jc * PROJ_J + OFF_V: jc * PROJ_J + OFF_V + 192]
                .bitcast(F32).rearrange("p (h d) -> p h d", h=H))
            nc.vector.tensor_copy(
                vvg_bf[:, base:base + H * EW]
                .rearrange("p (h w) -> p h w", h=H)[:, :, D:D + 24]
                .rearrange("p h (c v) -> p c h v", c=3),
                vg_nat[:, jc * 288:(jc + 1) * 288]
                .rearrange("p (c h v) -> p c h v", c=3, h=H))
            nc.gpsimd.memset(
                vvg_bf[:, base:base + H * EW]
                .rearrange("p (h w) -> p h w", h=H)[:, :, 40:41], 1.0)

        c_nat = ph0.tile([128, 4 * 12], F32R)
        for jc in range(4):
            sq = ph0b.tile([128, 144], F32, tag="sq", name="sq")
            nc.scalar.activation(sq[:], kg_nat[:, jc * 144:(jc + 1) * 144].bitcast(F32), AF.Square)
            red1 = ph0b.tile([128, 12 * 3], F32, tag="red1", name="red1")
            nc.vector.tensor_reduce(
                red1[:].rearrange("p (h c) -> p h c", h=H),
                sq[:].rearrange("p (c h q) -> p h c q", c=3, h=H),
                axis=AX.X, op=ALU.add)
            with nc.allow_low_precision(reason="f32r rounding of Cterm is fine"):
                nc.vector.tensor_reduce(
                    c_nat[:, jc * 12:(jc + 1) * 12].unsqueeze(-1),
                    red1[:].rearrange("p (h c) -> p h c", h=H),
                    axis=AX.X, op=ALU.add)

        def pe_T(dst_col_writes, src_ap):
            tp = ph0ps.tile([128, 128], F32R, tag="tps", name="tp")
            np_ = src_ap.shape[0]
            nf = src_ap.shape[1]
            nc.tensor.transpose(tp[0:nf, 0:np_], r32(src_ap),
                                ident_r[0:np_, 0:np_])
            for (dst, r0, r1) in dst_col_writes:
                if r0 % 64 == 0:
                    nc.vector.tensor_copy(dst, tp[r0:r1, 0:np_].bitcast(F32))
                else:
                    nc.scalar.copy(dst, tp[r0:r1, 0:np_].bitcast(F32))

        fTq = [ph0.tile([128, NI], F32R, tag=f"fTq{c}", name=f"fTq{c}")
               for c in range(3)]
        nc.gpsimd.memset(fTq[2][:].bitcast(F32), 0.0)
        nc.gpsimd.memset(fTq[2][96:108, :].bitcast(F32), 1.0)

        for jc in range(4):
            js = slice(jc * 128, (jc + 1) * 128)
            base = jc * PROJ_J
            pe_T([(kfullT[0][0:128, js], 0, 128)],
                 proj_nat[:, base + OFF_K: base + OFF_K + 128])
            pe_T([(kfullT[1][0:64, js], 0, 64)],
                 proj_nat[:, base + OFF_K + 128: base + OFF_K + 192])
            pe_T([(kfullT[1][64:128, js], 0, 64), (kfullT[2][0:64, js], 64, 128)],
                 kg_nat[:, jc * 144: jc * 144 + 128])
            pe_T([(kfullT[2][64:80, js], 0, 16)],
                 kg_nat[:, jc * 144 + 128: jc * 144 + 144])
            pe_T([(kfullT[2][96:108, js], 0, 12)], c_nat[:, jc * 12:(jc + 1) * 12])

        pe_T([(fTq[0][0:128, :], 0, 128)], projq[:, OFF_Q:OFF_Q + 128])
        pe_T([(fTq[1][0:64, :], 0, 64)], projq[:, OFF_Q + 128:OFF_Q + 192])
        pe_T([(fTq[1][64:128, :], 0, 64), (fTq[2][0:64, :], 64, 128)],
             qg_nat[:, 0:128])
        pe_T([(fTq[2][64:80, :], 0, 16)], qg_nat[:, 128:144])

        # qexp[c][p, (i,h)] = fTq[c][p,i] * qsel[p,(c,h)] via stride-0 APs
        for c in range(3):
            for half, eng in ((0, nc.vector), (1, nc.gpsimd)):
                i0_, i1_ = half * 64, (half + 1) * 64
                fv = fTq[c][:, i0_:i1_].bitcast(F32).unsqueeze(-1)
                qv = qsel_sb[:, c * H:(c + 1) * H].rearrange("p (a h) -> p a h", a=1)
                fb, qb = bass.broadcast_tensor_aps(fv, qv)
                eng.tensor_tensor(
                    qexp[c][:, i0_ * H:i1_ * H].rearrange("p (i e) -> p i e", e=H),
                    fb, qb, op=ALU.mult)
        if 'dbg_qexp0' in p:
            dq = ph0.tile([128, NI * H], F32)
            nc.vector.tensor_copy(dq[:], qexp[0][:].bitcast(F32))
            dma(p['dbg_qexp0'][:], dq[:])
            dk0 = ph0.tile([128, N], F32)
            nc.vector.tensor_copy(dk0[:], kfullT[0][:].bitcast(F32))
            dma(p['dbg_kf0'][:], dk0[:])
            dk2 = ph0.tile([128, N], F32)
            nc.vector.tensor_copy(dk2[:], kfullT[2][:].bitcast(F32))
            dma(p['dbg_kf2'][:], dk2[:])

    # =================== MAIN LOOP ===================
    with tc.tile_pool(name="znp", bufs=6) as znp, \
         tc.tile_pool(name="atp", bufs=6) as atp, \
         tc.tile_pool(name="smallp", bufs=3) as smallp, \
         tc.tile_pool(name="ps_zt", bufs=3, space="PSUM") as ps_zt, \
         tc.tile_pool(name="ps_lg", bufs=2, space="PSUM") as ps_lg, \
         tc.tile_pool(name="ps_sum", bufs=1, space="PSUM") as ps_sum, \
         tc.tile_pool(name="ps_op", bufs=1, space="PSUM") as ps_op, \
         tc.tile_pool(name="ps_epi", bufs=1, space="PSUM") as ps_epi:
        for sb in range(NSB):
            zt0 = z_tiles[sb * 2]
            zt1 = z_tiles[sb * 2 + 1]

            def zsl(q, jc):
                t = zt0 if q < 16 else zt1
                return t[:, (q % 16) * N + jc * 128:(q % 16) * N + (jc + 1) * 128]

            # logits + bias + exp + colsum per key chunk
            sums = ps_sum.tile([1, SBQ * H], F32, tag="sums", name="sums")
            attnT = []
            for jc in range(4):
                lg = ps_lg.tile([128, SBQ * H], F32, tag="lg", name="lg")
                for kc in range(3):
                    nc.tensor.matmul(
                        lg[:], r32(kfullT[kc][:, jc * 128:(jc + 1) * 128]),
                        r32(qexp[kc][:, sb * SBQ * H:(sb + 1) * SBQ * H]),
                        start=(kc == 0), stop=False)
                for q in range(SBQ):
                    nc.tensor.matmul(
                        lg[:, q * H:(q + 1) * H], zsl(q, jc), wbs_bf[:],
                        start=False, stop=True, skip_group_check=True)
                at = atp.tile([128, SBQ * H], BF16, tag="attnT", name="attnT")
                nc.scalar.activation(at[:], lg[:], AF.Exp)
                attnT.append(at)
                nc.tensor.matmul(sums[:], ones_bf[:], at[:],
                                 start=(jc == 0), stop=(jc == 3))

            # znat via PE transpose, 2 queries per PSUM bank
            znat = []
            for qq in range(SBQ // 2):
                ztp = ps_zt.tile([128, 2 * N], BF16, tag="ztp", name="ztp")
                for qi in range(2):
                    for jc in range(4):
                        nc.tensor.transpose(
                            ztp[:, qi * N + jc * 128: qi * N + (jc + 1) * 128],
                            zsl(qq * 2 + qi, jc), ident_bf[:])
                zn = znp.tile([128, 2 * N], BF16, tag="znat", name="znat")
                if qq % 4 == 3:
                    nc.scalar.copy(zn[:], ztp[:])
                else:
                    nc.vector.tensor_copy(zn[:], ztp[:])
                znat.append(zn)

            # normalization factors (off the PE critical path)
            rcp_row = smallp.tile([1, SBQ * H], F32, tag="rcp", name="rcp_row")
            nc.vector.reciprocal(rcp_row[:], sums[:])
            rcp_bc = smallp.tile([128, SBQ * H], F32, tag="rcpb", name="rcp_bc")
            nc.gpsimd.partition_broadcast(rcp_bc[:], rcp_row[:])

            # epi (out_scalar | rpg | sums), 32 i's, unnormalized attn
            epi = ps_epi.tile([32, H * EW], F32, tag="epi", name="epi")
            for h in range(H):
                for jc in range(4):
                    nc.tensor.matmul(
                        epi[:, h * EW:(h + 1) * EW],
                        attnT[jc][:].rearrange("p (q h) -> p h q", h=H)[:, h, :],
                        vvg_bf[:, (jc * H + h) * EW:(jc * H + h + 1) * EW],
                        start=(jc == 0), stop=(jc == 3))

            # out_pair: [d, (q,h)], accumulate over key chunks
            op_ps = ps_op.tile([128, SBQ * H], F32, tag="opps", name="op_ps")
            for qq in range(SBQ // 2):
                for qi in range(2):
                    q = qq * 2 + qi
                    for jc in range(4):
                        nc.tensor.matmul(
                            op_ps[:, q * H:(q + 1) * H],
                            znat[qq][:, qi * N + jc * 128: qi * N + (jc + 1) * 128],
                            attnT[jc][:, q * H:(q + 1) * H],
                            start=(jc == 0), stop=(jc == 3), skip_group_check=True)

            # epi extraction with per-(q,h) normalization
            if sb == 0 and 'dbg_at0' in p:
                da = smallp.tile([128, SBQ * H], F32, tag="dbg_a", name="dbg_a")
                nc.vector.tensor_copy(da[:], attnT[0][:])
                dma(p['dbg_at0'][:], da[:])
                dma(p['dbg_rcpb'][:], rcp_bc[:])
                de = smallp.tile([32, H * EW], F32, tag="dbg_e", name="dbg_e")
                nc.vector.tensor_copy(de[:], epi[:])
                dma(p['dbg_epi0'][:], de[:])
                do = smallp.tile([128, SBQ * H], F32, tag="dbg_o", name="dbg_o")
                nc.vector.tensor_copy(do[:], op_ps[:])
                dma(p['dbg_op0'][:], do[:])
                dz = smallp.tile([128, 2 * N], F32, tag="dbg_z", name="dbg_z")
                nc.vector.tensor_copy(dz[:], znat[0][:])
                dma(p['dbg_znat0'][:], dz[:])
            r0 = sb * SBQ
            rcp_qh = smallp.tile([32, H], F32, tag="rcpqh", name="rcp_qh")
            nc.vector.reciprocal(
                rcp_qh[:], epi[:].rearrange("p (h w) -> p w h", w=EW)[:, 40, :])
            rv = rcp_qh[:].unsqueeze(-1)
            ev_s = epi[:].rearrange("p (h w) -> p h w", h=H)[:, :, 0:D]
            rb, eb = bass.broadcast_tensor_aps(rv, ev_s)
            nc.vector.tensor_tensor(
                scalar_all[r0:r0 + SBQ, :].rearrange("p (h d) -> p h d", h=H),
                eb, rb, op=ALU.mult)
            ev_r = epi[:].rearrange("p (h w) -> p h w", h=H)[:, :, D:40] \
                .rearrange("p h (c v) -> p h c v", c=3)
            rb2, eb2 = bass.broadcast_tensor_aps(
                rcp_qh[:].unsqueeze(-1).unsqueeze(-1), ev_r)
            nc.vector.tensor_tensor(
                rpg_all[r0:r0 + SBQ, :].rearrange("p (c h v) -> p h c v", c=3, h=H),
                eb2, rb2, op=ALU.mult)

            # pairT with normalization folded into the PSUM->SBUF copy
            nc.vector.tensor_tensor(
                pairT[:].rearrange("p (h i) -> p h i", h=H)[:, :, r0:r0 + SBQ],
                op_ps[:].rearrange("p (q h) -> p h q", h=H),
                rcp_bc[:].rearrange("p (q h) -> p h q", h=H),
                op=ALU.mult)

    # =================== EPILOGUE ===================
    with tc.tile_pool(name="ep", bufs=1) as ep, \
         tc.tile_pool(name="eps", bufs=2, space="PSUM") as eps, \
         tc.tile_pool(name="ps_fin", bufs=1, space="PSUM") as ps_fin:
        wout_sb = ep.tile([128, 18 * C_S], BF16)
        dma(wout_sb[:], p['wout'][:])
        bout_f = ep.tile([1, C_S], F32)
        dma(bout_f[:], p['bout'][:])
        bout_bf = ep.tile([1, C_S], BF16)
        nc.vector.tensor_copy(bout_bf[:], bout_f[:])

        epsb = ep.tile([128, 1], F32)
        nc.gpsimd.memset(epsb[:], EPS)
        rpgm = ep.tile([128, 3 * 96], F32)
        for pp_ in range(3):
            nc.vector.tensor_scalar(rpgm[:, pp_ * 96:(pp_ + 1) * 96],
                                    rpg_all[:, pp_ * 96:(pp_ + 1) * 96],
                                    Rtq_sb[:, 9 + pp_: 9 + pp_ + 1], None,
                                    op0=ALU.subtract)
        rpl = ep.tile([128, 3 * 96], F32R)
        for o in range(3):
            d = rpl[:, o * 96:(o + 1) * 96]
            nc.vector.tensor_scalar(d, rpgm[:, 0:96], Rtq_sb[:, o:o + 1], None,
                                    op0=ALU.mult)
            for pp_ in (1, 2):
                nc.vector.scalar_tensor_tensor(
                    d, rpgm[:, pp_ * 96:(pp_ + 1) * 96],
                    Rtq_sb[:, pp_ * 3 + o: pp_ * 3 + o + 1], d.bitcast(F32),
                    op0=ALU.mult, op1=ALU.add)
        sq2 = ep.tile([128, 3 * 96], F32)
        nc.scalar.activation(sq2[:], rpl[:].bitcast(F32), AF.Square)
        nrm = ep.tile([128, 96], F32R)
        nc.vector.tensor_tensor(nrm[:], sq2[:, 0:96], sq2[:, 96:192], op=ALU.add)
        nc.vector.tensor_tensor(nrm[:], nrm[:].bitcast(F32), sq2[:, 192:288], op=ALU.add)
        nc.scalar.activation(nrm[:], nrm[:].bitcast(F32), AF.Sqrt, bias=epsb[:])

        fts = ep.tile([128, 6 * 128], BF16)

        def pe_T2_bf(dst, src_ap, rows):
            tp2 = eps.tile([128, 128], BF16, tag="tps2b", name="tp2b")
            nc.tensor.transpose(tp2[0:rows, :], src_ap, ident_bf[:])
            nc.vector.tensor_copy(dst, tp2[0:rows, :])

        def pe_T2_f(dst, src_ap, rows):
            tp2 = eps.tile([128, 128], F32R, tag="tps2f", name="tp2f")
            nc.tensor.transpose(tp2[0:rows, :], r32(src_ap), ident_r[:])
            nc.vector.tensor_copy(dst, tp2[0:rows, :].bitcast(F32))

        pe_T2_bf(fts[0:128, 0:128], scalar_all[:, 0:128], 128)
        pe_T2_bf(fts[0:64, 128:256], scalar_all[:, 128:192], 64)
        for o in range(3):
            pe_T2_f(fts[0:96, (2 + o) * 128:(2 + o) * 128 + 128],
                    rpl[:, o * 96:(o + 1) * 96], 96)
        pe_T2_f(fts[0:96, 5 * 128:5 * 128 + 128], nrm[:], 96)

        CH_ROWS = [128, 64, 96, 96, 96, 96] + [128] * 12
        fin = ps_fin.tile([128, C_S], F32)
        for c in range(18):
            rr = CH_ROWS[c]
            lhsT = fts[0:rr, c * 128:(c + 1) * 128] if c < 6 else \
                pairT[0:rr, (c - 6) * 128:(c - 6 + 1) * 128]
            nc.tensor.matmul(fin[:], lhsT, wout_sb[0:rr, c * C_S:(c + 1) * C_S],
                             start=(c == 0), stop=False)
        nc.tensor.matmul(fin[:], ones1_bf[:], bout_bf[:], start=False, stop=True)
        out_sb = ep.tile([128, C_S], F32)
        nc.vector.tensor_copy(out_sb[:], fin[:])
        dma(p['out'][:], out_sb[:])
        if 'dbg_pairT' in p:
            dp = ep.tile([128, H * 128], F32)
            nc.vector.tensor_copy(dp[:], pairT[:])
            dma(p['dbg_pairT'][:], dp[:])
            ds = ep.tile([128, H * D], F32)
            nc.vector.tensor_copy(ds[:], scalar_all[:])
            dma(p['dbg_sa'][:], ds[:])
            dma(p['dbg_rpg'][:], rpg_all[:])

    zp_cm.__exit__(None, None, None)
    pers_cm.__exit__(None, None, None)


# ======================= driver =======================
_NC_CACHE = {}


def _get_nc():
    if 'nc' not in _NC_CACHE:
        _NC_CACHE['nc'] = build_kernel()
    return _NC_CACHE['nc']


def kernel(**inputs):
    from concourse.bass_utils import run_bass_kernel_spmd
    inp = {k: np.asarray(v) for k, v in inputs.items()}
    packed = pack_weights(inp)
    in_maps, meta = [], []
    for core in range(N_CORES):
        m, b, i0 = per_core_inputs(inp, packed, core)
        in_maps.append(m)
        meta.append((b, i0))
    nc = _get_nc()
    res = run_bass_kernel_spmd(nc, in_maps, core_ids=list(range(N_CORES)))
    out = np.zeros((B, N, C_S), np.float32)
    for core in range(N_CORES):
        b, i0 = meta[core]
        out[b, i0:i0 + NI] = res.results[core]["out"]
    return out
